# revision 26
# baseline (speedup 1.0000x reference)
"""Trainium2 Bass kernel for MTGNN temporal classifier (single layer).

Self-contained: takes FULL inputs as numpy arrays, shards across 8 NeuronCores
(batch x time-half), runs one SPMD Bass/Tile program, gathers the full output.

Sharding: core = 2*b + th  (b in 0..3 batches, th in 0..1 time-halves).

Mixprop is computed via the powers-of-A refactor: since the channel conv1x1
commutes with node hops,  out = sum_k C_k A^k x  with C_k folded host-side.
The A^k are precomputed on host, scaled by 256 and quantized to fp8e4, so the
dominant N x N hop GEMMs run in fp8 DoubleRow (double-pumped) mode with
x (channel-major hcm transposed once per time step) as the fp8 stationary.
Hop outputs land directly channel-major, eliminating per-hop transposes and
blends.  LayerNorm is folded analytically into the skipE convolution; the
collectives are pairwise AllGathers: skip01 early, and rawE per row-chunk q
(overlapped with compute) with [stats] appended to the last one.
"""

import numpy as np
import ml_dtypes

import concourse.bass as bass
import concourse.tile as tile
import concourse.bass_isa as bass_isa
from concourse import bacc, mybir
from concourse import bass_utils

BF16 = mybir.dt.bfloat16
F32 = mybir.dt.float32
F8 = mybir.dt.float8e4
bf16 = ml_dtypes.bfloat16
f8 = ml_dtypes.float8_e4m3
AF = mybir.ActivationFunctionType
ALU = mybir.AluOpType
DR = mybir.MatmulPerfMode.DoubleRow

# problem dims
B, C_IN, N, T = 4, 129, 1259, 25
RC, CC, SC, EC, OUT = 128, 126, 128, 128, 64
K = 3
T1 = T - (K - 1)          # 23
NP = 1280                 # padded node count
NV = NP // 128            # 10 node blocks
TAU = 12                  # local output time steps per core (incl. 1 pad on th=1)
TLOC = TAU + 2            # 14 local input time steps
VCH = [(0, 512), (512, 512), (1024, 256)]   # v chunks (full NP)
CNT = float(RC * N * T1)  # layernorm element count per batch
EPS = 1e-5
NQ = 3                    # row-chunk count (4 tau each)
SADJ = 256.0              # fp8 scale on A^k

_CACHE = {}


def _build_program(debug_taps=False):
    nc = bacc.Bacc("TRN2", target_bir_lowering=False, debug=False, num_devices=8)

    def din(name, shape, dt=BF16):
        return nc.dram_tensor(name, shape, dt, kind="ExternalInput").ap()

    x_hi = din("x_hi", [128, TLOC, NP])
    x_lo = din("x_lo", [TLOC, NP])          # channel 128, [t, v]
    gk_d = [[din(f"g{mp}{k}", [128, NV, NP], F8) for k in range(3)]
            for mp in range(2)]             # SADJ * (A^k).T padded, fp8
    wsT_hi = din("wsT_hi", [128, 128])
    wsT_lo = din("wsT_lo", [1, 128])
    w0T_hi = din("w0T_hi", [128, TLOC, 128])
    w0T_lo = din("w0T_lo", [TLOC, 128])
    wfT = din("wfT", [128, K, CC])
    wgT = din("wgT", [128, K, CC])
    bf_v = din("bf_v", [CC, 1], F32)
    bg_v = din("bg_v", [CC, 1], F32)
    w1T = din("w1T", [CC, TAU, 128])
    wCT = din("wCT", [128, 8, 128])         # folded conv mats, transposed
    b_resid_v = din("b_resid_v", [128, 1], F32)
    wET = din("wET", [128, TAU, 128])
    wEsum_v = din("wEsum_v", [128, 1], F32)
    b01_v = din("b01_v", [128, 1], F32)
    we1T = din("we1T", [128, 128])
    be1_v = din("be1_v", [128, 1], F32)
    we2T = din("we2T", [128, OUT])
    be2_v = din("be2_v", [OUT, 1], F32)
    whT = din("whT", [OUT, 1])
    bh_v = din("bh_v", [1, 1], F32)
    tmask = din("tmask", [128, TAU], F32)
    y = nc.dram_tensor("y", [1, NP], F32, kind="ExternalOutput").ap()
    taps = {}
    if debug_taps:
        for nm, shp, dt in [("d_hcm", [128, TAU, NP], BF16),
                            ("d_x8", [128, TAU, NV, 128], F8),
                            ("d_macc", [128, TAU, NP], BF16),
                            ("d_skip", [128, NP], F32),
                            ("d_rawE", [128, NP + 2], F32)]:
            taps[nm] = nc.dram_tensor(nm, shp, dt, kind="ExternalOutput").ap()

    with tile.TileContext(nc) as tc:
        with (
            tc.tile_pool(name="persist", bufs=1) as pp,
            tc.tile_pool(name="dram", bufs=1, space="DRAM") as dram,
        ):
            # ---- persistent tiles ----
            hcm = pp.tile([128, TAU, NP], BF16)       # f*g, channel-major (rows 126,127 zero)
            macc = pp.tile([128, TAU, NP], BF16)      # residual + mixprop accumulator
            skip_acc = pp.tile([128, NP], F32)        # skip0+skip1 partial
            rawE_sum = pp.tile([128, NP + 2], F32)    # combined rawE | stats
            x8_all = pp.tile([128, TAU, NV, 128], F8) # hcm transposed, fp8
            g00_t = pp.tile([128, NV, NP], F8)        # first hop matrix, preloaded
            w1T_t = pp.tile([CC, TAU, 128], BF16)
            wC_t = pp.tile([128, 8, 128], BF16)
            wET_t = pp.tile([128, TAU, 128], BF16)
            brv = pp.tile([128, 1], F32)
            wEs_t = pp.tile([128, 1], F32)
            b01_t = pp.tile([128, 1], F32)
            we1_t = pp.tile([128, 128], BF16)
            be1_t = pp.tile([128, 1], F32)
            we2_t = pp.tile([128, OUT], BF16)
            be2_t = pp.tile([OUT, 1], F32)
            whT_t = pp.tile([OUT, 1], BF16)
            bh_t = pp.tile([1, 1], F32)
            tmask_t = pp.tile([128, TAU], F32)
            sums_t = pp.tile([128, TAU], F32)
            sqs_t = pp.tile([128, TAU], F32)
            stats_p = pp.tile([128, 2], F32)
            ones_t = pp.tile([128, 128], F32)

            nc.gpsimd.dma_start(g00_t[:], gk_d[0][0][:])
            nc.vector.memset(hcm[:], 0.0)
            nc.vector.memset(macc[:, :, N:NP], 0.0)
            nc.vector.memset(ones_t[:], 1.0)

            # collective DRAM staging
            cc1_in = dram.tile([128, NP], F32)
            cc1_out = dram.tile([256, NP], F32)
            ccq_in = [dram.tile([128, NP + 2], F32, name=f"ccqi{q}")
                      for q in range(NQ)]
            ccq_out = [dram.tile([256, NP + 2], F32, name=f"ccqo{q}")
                       for q in range(NQ)]

            # ================= stage A =================
            with tc.tile_pool(name="stageA", bufs=1) as pa, \
                 tc.tile_pool(name="stag", bufs=2) as pstag:
                xh = pa.tile([128, TLOC, NP], BF16)
                H0 = pa.tile([128, TLOC, NP], BF16)
                xlo14 = pa.tile([TLOC, NP], BF16)       # [t, v] on 14 partitions
                ws_hi_t = pa.tile([128, 128], BF16)
                ws_lo_t = pa.tile([1, 128], BF16)
                w0_hi_t = pa.tile([128, TLOC, 128], BF16)
                w0_lo_t = pa.tile([TLOC, 128], BF16)
                wf_t = pa.tile([128, K, CC], BF16)
                wg_t = pa.tile([128, K, CC], BF16)
                bfv_t = pa.tile([CC, 1], F32)
                bgv_t = pa.tile([CC, 1], F32)
                for t_, d_ in [(ws_hi_t, wsT_hi), (ws_lo_t, wsT_lo),
                               (wf_t, wfT), (wg_t, wgT), (bfv_t, bf_v),
                               (bgv_t, bg_v), (w0_hi_t, w0T_hi),
                               (w0_lo_t, w0T_lo), (xlo14, x_lo)]:
                    nc.sync.dma_start(t_[:], d_[:])
                for tp_ in range(TLOC):
                    nc.sync.dma_start(xh[:, tp_, :], x_hi[:, tp_, :])

                for t_, d_ in [(w1T_t, w1T), (wC_t, wCT),
                               (wET_t, wET), (brv, b_resid_v), (wEs_t, wEsum_v),
                               (b01_t, b01_v), (we1_t, we1T), (be1_t, be1_v),
                               (we2_t, we2T), (be2_t, be2_v), (whT_t, whT),
                               (bh_t, bh_v), (tmask_t, tmask)]:
                    nc.gpsimd.dma_start(t_[:], d_[:])

                # start conv (H0) interleaved with filt/gate, per t'.
                # Interleaving keeps the PE continuously fed (p-state ramp).
                with tc.tile_pool(name="psA1", bufs=3, space="PSUM") as psA1, \
                     tc.tile_pool(name="psA2", bufs=4, space="PSUM") as psA2:

                    def fg_step(tau):
                        fs = pstag.tile([CC, NP], BF16, tag="fs")
                        gs = pstag.tile([CC, NP], BF16, tag="gs")
                        for dst, w_t, func, bias in ((fs, wf_t, AF.Tanh, bfv_t),
                                                     (gs, wg_t, AF.Sigmoid,
                                                      bgv_t)):
                            for vo, vl in VCH:
                                psb = psA2.tile([CC, 512], F32, tag="ps_fg")
                                for k in range(K):
                                    nc.tensor.matmul(psb[:, 0:vl], w_t[:, k, :],
                                                     H0[:, tau + k, vo:vo + vl],
                                                     start=(k == 0),
                                                     stop=(k == K - 1))
                                nc.scalar.activation(dst[:, vo:vo + vl],
                                                     psb[:, 0:vl], func,
                                                     bias=bias[:], scale=1.0)
                        nc.vector.tensor_tensor(hcm[0:CC, tau, :], fs[:], gs[:],
                                                op=ALU.mult)

                    for tp_ in range(TLOC):
                        stg = pstag.tile([1, NP], BF16, tag="xlo_stage")
                        nc.sync.dma_start(stg[:], x_lo[tp_:tp_ + 1, :])
                        for i, (vo, vl) in enumerate(VCH):
                            psum = psA1.tile([128, 512], F32, tag="ps_start")
                            nc.tensor.matmul(psum[:, 0:vl], ws_hi_t[:],
                                             xh[:, tp_, vo:vo + vl],
                                             start=True, stop=False)
                            nc.tensor.matmul(psum[:, 0:vl], ws_lo_t[:],
                                             stg[:, vo:vo + vl],
                                             start=False, stop=True)
                            if i % 2 == 0:
                                nc.vector.tensor_copy(H0[:, tp_, vo:vo + vl],
                                                      psum[:, 0:vl])
                            else:
                                nc.scalar.activation(H0[:, tp_, vo:vo + vl],
                                                     psum[:, 0:vl], AF.Copy)
                        if tp_ >= K - 1:
                            tau = tp_ - (K - 1)
                            fg_step(tau)
                            # residual (+ biases) into macc for this tau, on
                            # the otherwise-idle gpsimd engine so H0's space
                            # frees promptly for mixprop tiles
                            nc.gpsimd.tensor_scalar_add(macc[:, tau, 0:N],
                                                        H0[:, tau + 2, 0:N],
                                                        brv[:])
                    # skip0: contract (c, t) chunk-sequential; c=128 via K=14
                    for i, (vo, vl) in enumerate(VCH):
                        s0ps = psA1.tile([128, 512], F32, tag="ps_start",
                                         name=f"s0ps{i}")
                        for tp_ in range(TLOC):
                            nc.tensor.matmul(s0ps[:, 0:vl], w0_hi_t[:, tp_, :],
                                             xh[:, tp_, vo:vo + vl],
                                             start=(tp_ == 0), stop=False)
                        nc.tensor.matmul(s0ps[:, 0:vl], w0_lo_t[:],
                                         xlo14[:, vo:vo + vl],
                                         start=False, stop=True)
                        nc.vector.tensor_copy(skip_acc[:, vo:vo + vl],
                                              s0ps[:, 0:vl])

                # skip1 conv partial (contract c,tau over local range)
                with tc.tile_pool(name="psA2s", bufs=2, space="PSUM") as psA2s:
                    for vo, vl in VCH:
                        psum = psA2s.tile([128, 512], F32, tag="ps_s1")
                        for tau in range(TAU):
                            nc.tensor.matmul(psum[:, 0:vl], w1T_t[:, tau, :],
                                             hcm[0:CC, tau, vo:vo + vl],
                                             start=(tau == 0), stop=(tau == TAU - 1))
                        nc.vector.tensor_tensor(skip_acc[:, vo:vo + vl],
                                                skip_acc[:, vo:vo + vl],
                                                psum[:, 0:vl], op=ALU.add)

            # ================= mixprop (powers of A, fp8 DoubleRow) ========
            with tc.tile_pool(name="mxg", bufs=1) as mxg, \
                 tc.tile_pool(name="mxu", bufs=1) as mxu, \
                 tc.tile_pool(name="mxr", bufs=2) as mxr, \
                 tc.tile_pool(name="mxT", bufs=2) as mxT, \
                 tc.tile_pool(name="psU", bufs=2, space="PSUM") as psU, \
                 tc.tile_pool(name="psC", bufs=2, space="PSUM") as psC:
                # g slot loads for mp=0 (k=1,2) BEFORE the cc1 collective so the
                # transfers overlap it on the in-order gpsimd queue
                g12_mp0 = []
                for k in (1, 2):
                    g = mxg.tile([128, NV, NP], F8, tag=f"g{k}", name=f"g{k}_0")
                    nc.gpsimd.dma_start(g[:], gk_d[0][k][:])
                    g12_mp0.append(g)

                for mp in range(2):
                    if mp == 0:
                        gs_t = [g00_t] + g12_mp0
                    else:
                        gs_t = []
                        for k in range(3):
                            g = mxg.tile([128, NV, NP], F8, tag=f"g{k}",
                                         name=f"g{k}_1")
                            nc.gpsimd.dma_start(g[:], gk_d[1][k][:])
                            gs_t.append(g)
                    for q in range(NQ):
                        u8 = mxu.tile([128, 3, 4, NP], BF16, tag="u8",
                                      name=f"u8_{mp}_{q}")
                        if mp == 0:
                            for ti in range(4):
                                t = 4 * q + ti
                                xT = mxT.tile([128, NV, 128], BF16, tag="xT")
                                nc.sync.dma_start_transpose(xT[:], hcm[:, t, :])
                                if ti % 2 == 0:
                                    nc.scalar.activation(x8_all[:, t, :, :],
                                                         xT[:], AF.Copy)
                                else:
                                    nc.vector.tensor_copy(x8_all[:, t, :, :],
                                                          xT[:])
                        for k in range(3):
                            for ti in range(4):
                                t = 4 * q + ti
                                pu = psU.tile([128, NP], F32, tag="pu")
                                for vo, vl in VCH:
                                    for j in range(5):
                                        nc.tensor.matmul(
                                            pu[:, vo:vo + vl],
                                            x8_all[:, t, 2 * j:2 * j + 2, :],
                                            gs_t[k][:, 2 * j:2 * j + 2, vo:vo + vl],
                                            start=(j == 0), stop=(j == 4),
                                            perf_mode=DR)
                                if (k + ti) % 2 == 0:
                                    nc.vector.tensor_copy(u8[:, k, ti, :], pu[:])
                                else:
                                    nc.scalar.activation(u8[:, k, ti, :], pu[:],
                                                         AF.Copy)
                        # conv1x1: C0 hcm + sum_k Ck u_k, accumulate into macc
                        for ti in range(4):
                            t = 4 * q + ti
                            for ci, (vo, vl) in enumerate(VCH):
                                pc = psC.tile([128, 512], F32, tag="pc")
                                nc.tensor.matmul(pc[:, 0:vl], wC_t[:, 4 * mp, :],
                                                 hcm[:, t, vo:vo + vl],
                                                 start=True, stop=False)
                                for k in range(3):
                                    nc.tensor.matmul(
                                        pc[:, 0:vl], wC_t[:, 4 * mp + 1 + k, :],
                                        u8[:, k, ti, vo:vo + vl],
                                        start=False, stop=(k == 2))
                                hi = min(vo + vl, N)
                                nc.vector.tensor_tensor(
                                    macc[:, t, vo:hi], macc[:, t, vo:hi],
                                    pc[:, 0:hi - vo], op=ALU.add)
                        if mp == 1:
                            # macc rows of q final: stats + rawE_q + collective
                            for ti in range(4):
                                t = 4 * q + ti
                                nc.vector.reduce_sum(sums_t[:, t:t + 1],
                                                     macc[:, t, :],
                                                     axis=mybir.AxisListType.X)
                                scr = mxT.tile([128, NP], BF16, tag="sq_scr")
                                nc.scalar.activation(scr[:], macc[:, t, :],
                                                     AF.Square,
                                                     accum_out=sqs_t[:, t:t + 1])
                            rq = mxr.tile([128, NP], F32, tag="rq")
                            for vo, vl in VCH:
                                psum = psC.tile([128, 512], F32, tag="pc",
                                                name="ps_rEq")
                                for ti in range(4):
                                    nc.tensor.matmul(
                                        psum[:, 0:vl], wET_t[:, 4 * q + ti, :],
                                        macc[:, 4 * q + ti, vo:vo + vl],
                                        start=(ti == 0), stop=(ti == 3))
                                nc.vector.tensor_copy(rq[:, vo:vo + vl],
                                                      psum[:, 0:vl])
                            nc.gpsimd.dma_start(ccq_in[q][:, 0:NP], rq[:])
                            if q == NQ - 1:
                                # layernorm partial stats appended to last cc
                                msum = mxr.tile([128, TAU], F32, tag="msum")
                                nc.vector.tensor_tensor(msum[:], sums_t[:],
                                                        tmask_t[:], op=ALU.mult)
                                nc.vector.reduce_sum(stats_p[:, 0:1], msum[:],
                                                     axis=mybir.AxisListType.X)
                                nc.vector.tensor_tensor(msum[:], sqs_t[:],
                                                        tmask_t[:], op=ALU.mult)
                                nc.vector.reduce_sum(stats_p[:, 1:2], msum[:],
                                                     axis=mybir.AxisListType.X)
                                nc.gpsimd.dma_start(ccq_in[q][:, NP:NP + 2],
                                                    stats_p[:])
                            nc.gpsimd.collective_compute(
                                "AllGather", ALU.bypass,
                                ins=[ccq_in[q].opt()], outs=[ccq_out[q].opt()],
                                replica_groups=[[0, 1], [2, 3], [4, 5], [6, 7]])
                            # combine halves (accumulating across q too)
                            if q == 0:
                                nc.gpsimd.dma_start(rawE_sum[:, 0:NP],
                                                    ccq_out[q][0:128, 0:NP])
                            else:
                                nc.gpsimd.dma_start(rawE_sum[:, 0:NP],
                                                    ccq_out[q][0:128, 0:NP],
                                                    accum_op=ALU.add)
                            nc.gpsimd.dma_start(rawE_sum[:, 0:NP],
                                                ccq_out[q][128:256, 0:NP],
                                                accum_op=ALU.add)
                            if q == NQ - 1:
                                nc.gpsimd.dma_start(
                                    rawE_sum[:, NP:NP + 2],
                                    ccq_out[q][0:128, NP:NP + 2])
                                nc.gpsimd.dma_start(
                                    rawE_sum[:, NP:NP + 2],
                                    ccq_out[q][128:256, NP:NP + 2],
                                    accum_op=ALU.add)
                    if mp == 0:
                        # pairwise AllGather of skip01 partials, emitted after
                        # mp=0 so its sync-engine completion wait sits behind
                        # the hcm transposes in the sync queue
                        nc.gpsimd.dma_start(cc1_in[:], skip_acc[:])
                        nc.gpsimd.collective_compute(
                            "AllGather", ALU.bypass,
                            ins=[cc1_in.opt()], outs=[cc1_out.opt()],
                            replica_groups=[[0, 1], [2, 3], [4, 5], [6, 7]])

            if debug_taps:
                nc.gpsimd.dma_start(taps["d_hcm"][:], hcm[:])
                nc.gpsimd.dma_start(taps["d_x8"][:], x8_all[:])
                nc.gpsimd.dma_start(taps["d_macc"][:], macc[:])
                nc.gpsimd.dma_start(taps["d_skip"][:], skip_acc[:])
            # ================= layernorm scalars + end stage =================
            with tc.tile_pool(name="late", bufs=1) as pl, \
                 tc.tile_pool(name="psL", bufs=1, space="PSUM") as ps:
                if debug_taps:
                    nc.gpsimd.dma_start(taps["d_rawE"][:], rawE_sum[:])
                # combine skip AllGather halves
                nc.gpsimd.dma_start(skip_acc[:], cc1_out[0:128, :])
                nc.gpsimd.dma_start(skip_acc[:], cc1_out[128:256, :],
                                    accum_op=ALU.add)

                # layernorm scalars: partition sum broadcast via ones-matmul
                st_r = pl.tile([128, 2], F32)
                pst = ps.tile([128, 2], F32, tag="ps_st")
                nc.tensor.matmul(pst[:], ones_t[:], rawE_sum[:, NP:NP + 2],
                                 start=True, stop=True)
                nc.vector.tensor_copy(st_r[:], pst[:])
                mv = pl.tile([128, 1], F32)
                msqv = pl.tile([128, 1], F32)
                varv = pl.tile([128, 1], F32)
                m2v = pl.tile([128, 1], F32)
                svv = pl.tile([128, 1], F32)
                rv = pl.tile([128, 1], F32)
                rmv = pl.tile([128, 1], F32)
                bias_c = pl.tile([128, 1], F32)
                nc.vector.tensor_scalar_mul(mv[:], st_r[:, 0:1], 1.0 / CNT)
                nc.vector.tensor_scalar_mul(msqv[:], st_r[:, 1:2], 1.0 / CNT)
                nc.vector.tensor_tensor(m2v[:], mv[:], mv[:], op=ALU.mult)
                nc.vector.tensor_scalar(varv[:], msqv[:], m2v[:], EPS,
                                        op0=ALU.subtract, op1=ALU.add)
                nc.scalar.sqrt(svv[:], varv[:])
                nc.vector.reciprocal(rv[:], svv[:])
                nc.vector.tensor_scalar(rmv[:], rv[:], mv[:], -1.0,
                                        op0=ALU.mult, op1=ALU.mult)
                # bias_c = b01 - r*m*wEsum
                nc.vector.scalar_tensor_tensor(bias_c[:], wEs_t[:], rmv[:],
                                               b01_t[:], ALU.mult, ALU.add)
                # skip_pre = skip01 + r*rawE ; relu with bias
                skip_pre = pl.tile([128, NP], F32)
                nc.vector.scalar_tensor_tensor(skip_pre[:], rawE_sum[:, 0:NP],
                                               rv[:], skip_acc[:],
                                               ALU.mult, ALU.add)
                rsk = pl.tile([128, NP], BF16)
                nc.vector.tensor_scalar(rsk[:], skip_pre[:], bias_c[:], 0.0,
                                        op0=ALU.add, op1=ALU.max)

                # end convs + head, chunk-pipelined across engines
                o1 = pl.tile([128, NP], BF16)
                o2 = pl.tile([OUT, NP], BF16)
                y_sb = pl.tile([1, NP], F32)
                ps1 = ps.tile([128, 1536], F32, tag="ps_e1")
                ps2 = ps.tile([OUT, 1536], F32, tag="ps_e2")
                psh = ps.tile([1, 1536], F32, tag="ps_e1", name="psh")
                for vo, vl in VCH:
                    nc.tensor.matmul(ps1[:, vo:vo + vl], we1_t[:],
                                     rsk[:, vo:vo + vl], start=True, stop=True)
                    nc.scalar.activation(o1[:, vo:vo + vl], ps1[:, vo:vo + vl],
                                         AF.Relu, bias=be1_t[:], scale=1.0)
                    nc.tensor.matmul(ps2[:, vo:vo + vl], we2_t[:],
                                     o1[:, vo:vo + vl], start=True, stop=True)
                    nc.vector.tensor_scalar_add(o2[:, vo:vo + vl],
                                                ps2[:, vo:vo + vl], be2_t[:])
                    nc.tensor.matmul(psh[:, vo:vo + vl], whT_t[:],
                                     o2[:, vo:vo + vl], start=True, stop=True)
                nc.scalar.activation(y_sb[:], psh[:, 0:NP], AF.Sigmoid,
                                     bias=bh_t[:], scale=1.0)
                nc.gpsimd.dma_start(y[:], y_sb[:])

    nc.compile()
    return nc


def _norm_adj(a):
    an = a + np.eye(N, dtype=np.float64)
    return an / an.sum(axis=1, keepdims=True)


def _gk_pack(ak):
    """SADJ * (A^k).T zero-padded to [NP, NP] -> [128, NV, NP] fp8."""
    gp = np.zeros((NP, NP), dtype=np.float32)
    gp[:N, :N] = SADJ * ak.T.astype(np.float32)
    gp = np.clip(gp, -240, 240)
    return gp.reshape(NV, 128, NP).transpose(1, 0, 2).astype(f8)


def _coefs(w):
    W = [w[:, i * CC:(i + 1) * CC] for i in range(4)]
    C0 = W[0] + 0.5 * (W[1] + W[2] + W[3])
    C1 = 0.5 * W[1] + 0.25 * W[2] + 0.125 * W[3]
    C2 = 0.25 * W[2] + 0.125 * W[3]
    C3 = 0.125 * W[3]
    return C0, C1, C2, C3


def _prep_inputs(inputs):
    x = np.asarray(inputs["x"], np.float32)
    adj = np.asarray(inputs["adj"], np.float32)
    w_start = np.asarray(inputs["w_start"], np.float32)
    b_start = np.asarray(inputs["b_start"], np.float32)
    w_filt = np.asarray(inputs["w_filt"], np.float32)[:, :, 0, :]
    b_filt = np.asarray(inputs["b_filt"], np.float32)
    w_gate = np.asarray(inputs["w_gate"], np.float32)[:, :, 0, :]
    b_gate = np.asarray(inputs["b_gate"], np.float32)
    w_skip0 = np.asarray(inputs["w_skip0"], np.float32)[:, :, 0, :]
    b_skip0 = np.asarray(inputs["b_skip0"], np.float32)
    w_skip1 = np.asarray(inputs["w_skip1"], np.float32)[:, :, 0, :]
    b_skip1 = np.asarray(inputs["b_skip1"], np.float32)
    w_mp1 = np.asarray(inputs["w_mp1"], np.float32)
    b_mp1 = np.asarray(inputs["b_mp1"], np.float32)
    w_mp2 = np.asarray(inputs["w_mp2"], np.float32)
    b_mp2 = np.asarray(inputs["b_mp2"], np.float32)
    w_skipE = np.asarray(inputs["w_skipE"], np.float32)[:, :, 0, :]
    b_skipE = np.asarray(inputs["b_skipE"], np.float32)
    w_end1 = np.asarray(inputs["w_end1"], np.float32)
    b_end1 = np.asarray(inputs["b_end1"], np.float32)
    w_end2 = np.asarray(inputs["w_end2"], np.float32)
    b_end2 = np.asarray(inputs["b_end2"], np.float32)
    w_head = np.asarray(inputs["w_head"], np.float32)
    b_head = np.asarray(inputs["b_head"], np.float32)

    shared = {}
    for mp, a0 in ((0, adj), (1, adj.T)):
        an = _norm_adj(a0.astype(np.float64))
        ak = an
        for k in range(3):
            shared[f"g{mp}{k}"] = _gk_pack(ak)
            if k < 2:
                ak = ak @ an

    # folded conv1x1 matrices: wCT[:, 4*mp + k, :] = C_k(mp).T (scaled)
    wC = np.zeros((128, 8, 128), np.float32)
    for mp, w in ((0, w_mp1), (1, w_mp2)):
        C = _coefs(w)
        wC[:CC, 4 * mp + 0, :] = C[0].T
        for k in range(1, 4):
            wC[:CC, 4 * mp + k, :] = C[k].T / SADJ
    shared["wCT"] = wC.astype(bf16)

    wsT = w_start.T  # [129, 128]
    shared.update({
        "wsT_hi": wsT[:128].astype(bf16),
        "wsT_lo": wsT[128:129].astype(bf16),
        "wfT": w_filt.transpose(1, 2, 0).astype(bf16),
        "wgT": w_gate.transpose(1, 2, 0).astype(bf16),
        "bf_v": (b_filt + w_filt.sum(2) @ b_start).reshape(CC, 1).astype(np.float32),
        "bg_v": (b_gate + w_gate.sum(2) @ b_start).reshape(CC, 1).astype(np.float32),
        "b_resid_v": (b_start + b_mp1 + b_mp2).reshape(128, 1).astype(np.float32),
        "wEsum_v": w_skipE.sum((1, 2)).reshape(128, 1).astype(np.float32),
        "b01_v": (b_skip0 + b_skip1 + b_skipE).reshape(128, 1).astype(np.float32),
        "we1T": w_end1.T.astype(bf16),
        "be1_v": b_end1.reshape(128, 1).astype(np.float32),
        "we2T": w_end2.T.astype(bf16),
        "be2_v": b_end2.reshape(OUT, 1).astype(np.float32),
        "whT": w_head.T.astype(bf16),
        "bh_v": b_head.reshape(1, 1).astype(np.float32),
    })

    in_maps = []
    for core in range(8):
        b, th = core // 2, core % 2
        t_lo = 0 if th == 0 else TAU
        # x slice [129, 1280, TLOC] zero-padded in nodes and t
        xp = np.zeros((C_IN, TLOC, NP), np.float32)
        t_hi = min(t_lo + TLOC, T)
        xp[:, 0:t_hi - t_lo, :N] = x[b, :, :, t_lo:t_hi].transpose(0, 2, 1)
        # skip0 weight slots aligned to local t: core owns t range
        w0T = np.zeros((C_IN, TLOC, 128), np.float32)
        own_lo, own_hi = (0, 13) if th == 0 else (13, T)
        for tp_ in range(TLOC):
            tg = t_lo + tp_
            if own_lo <= tg < own_hi:
                w0T[:, tp_, :] = w_skip0[:, :, tg].T
        # skip1 / skipE weight slots aligned to local tau
        w1Ta = np.zeros((CC, TAU, 128), np.float32)
        wETa = np.zeros((128, TAU, 128), np.float32)
        for tau in range(TAU):
            tg = t_lo + tau
            if tg < T1:
                w1Ta[:, tau, :] = w_skip1[:, :, tg].T
                wETa[:, tau, :] = w_skipE[:, :, tg].T
        tm = np.ones((128, TAU), np.float32)
        if th == 1:
            tm[:, T1 - TAU:] = 0.0  # tau slots beyond T1 are padding
        m = dict(shared)
        m["x_hi"] = xp[:128].astype(bf16)
        m["x_lo"] = xp[128].astype(bf16)
        m["w0T_hi"] = w0T[:128].astype(bf16)
        m["w0T_lo"] = w0T[128].astype(bf16)
        m["w1T"] = w1Ta.astype(bf16)
        m["wET"] = wETa.astype(bf16)
        m["tmask"] = tm
        in_maps.append(m)
    return in_maps


def kernel(**inputs):
    if "nc" not in _CACHE:
        _CACHE["nc"] = _build_program()
    nc = _CACHE["nc"]
    in_maps = _prep_inputs(inputs)
    res = bass_utils.run_bass_kernel_spmd(nc, in_maps, core_ids=list(range(8)))
    out = np.empty((B, N), np.float32)
    for b in range(B):
        out[b] = res.results[2 * b]["y"][0, :N]
    return out


# revision 27
# speedup vs baseline: 1.3103x; 1.3103x over previous
"""Trainium2 Bass kernel for MTGNN temporal classifier (single layer).

Self-contained: takes FULL inputs as numpy arrays, shards across 8 NeuronCores
(batch x time-half), runs one SPMD Bass/Tile program, gathers the full output.

Sharding: core = 2*b + th  (b in 0..3 batches, th in 0..1 time-halves).

Mixprop is computed via the powers-of-A refactor: since the channel conv1x1
commutes with node hops,  out = sum_k C_k A^k x  with C_k folded host-side.
The A^k are precomputed on host, scaled by 256 and quantized to fp8e4, so the
dominant N x N hop GEMMs run in fp8 DoubleRow (double-pumped) mode with
x (channel-major hcm transposed once per time step) as the fp8 stationary.
Hop outputs land directly channel-major, eliminating per-hop transposes and
blends.  LayerNorm is folded analytically into the skipE convolution; the
collectives are pairwise AllGathers: skip01 early, and rawE per row-chunk q
(overlapped with compute) with [stats] appended to the last one.
"""

import numpy as np
import ml_dtypes

import concourse.bass as bass
import concourse.tile as tile
import concourse.bass_isa as bass_isa
from concourse import bacc, mybir
from concourse import bass_utils

BF16 = mybir.dt.bfloat16
F32 = mybir.dt.float32
F8 = mybir.dt.float8e4
bf16 = ml_dtypes.bfloat16
f8 = ml_dtypes.float8_e4m3
AF = mybir.ActivationFunctionType
ALU = mybir.AluOpType
DR = mybir.MatmulPerfMode.DoubleRow

# problem dims
B, C_IN, N, T = 4, 129, 1259, 25
RC, CC, SC, EC, OUT = 128, 126, 128, 128, 64
K = 3
T1 = T - (K - 1)          # 23
NP = 1280                 # padded node count
NV = NP // 128            # 10 node blocks
TAU = 12                  # local output time steps per core (incl. 1 pad on th=1)
TLOC = TAU + 2            # 14 local input time steps
VCH = [(0, 512), (512, 512), (1024, 256)]   # v chunks (full NP)
CNT = float(RC * N * T1)  # layernorm element count per batch
EPS = 1e-5
NQ = 3                    # row-chunk count (4 tau each)
SADJ = 256.0              # fp8 scale on A^k

_CACHE = {}


def _build_program(debug_taps=False):
    nc = bacc.Bacc("TRN2", target_bir_lowering=False, debug=False, num_devices=8)

    def din(name, shape, dt=BF16):
        return nc.dram_tensor(name, shape, dt, kind="ExternalInput").ap()

    x_hi = din("x_hi", [128, TLOC, NP])
    x_lo = din("x_lo", [TLOC, NP])          # channel 128, [t, v]
    gk_d = [[din(f"g{mp}{k}", [128, NV, NP], F8) for k in range(3)]
            for mp in range(2)]             # SADJ * (A^k).T padded, fp8
    wsT_hi = din("wsT_hi", [128, 128])
    wsT_lo = din("wsT_lo", [1, 128])
    w0T_hi = din("w0T_hi", [128, TLOC, 128])
    w0T_lo = din("w0T_lo", [TLOC, 128])
    wfT = din("wfT", [128, K, CC])
    wgT = din("wgT", [128, K, CC])
    bf_v = din("bf_v", [CC, 1], F32)
    bg_v = din("bg_v", [CC, 1], F32)
    w1T = din("w1T", [CC, TAU, 128])
    wCT = din("wCT", [128, 8, 128])         # folded conv mats, transposed
    b_resid_v = din("b_resid_v", [128, 1], F32)
    wET = din("wET", [128, TAU, 128])
    wEsum_v = din("wEsum_v", [128, 1], F32)
    b01_v = din("b01_v", [128, 1], F32)
    we1T = din("we1T", [128, 128])
    be1_v = din("be1_v", [128, 1], F32)
    we2T = din("we2T", [128, OUT])
    be2_v = din("be2_v", [OUT, 1], F32)
    whT = din("whT", [OUT, 1])
    bh_v = din("bh_v", [1, 1], F32)
    tmask = din("tmask", [128, TAU], F32)
    y = nc.dram_tensor("y", [1, NP], F32, kind="ExternalOutput").ap()
    taps = {}
    if debug_taps:
        for nm, shp, dt in [("d_hcm", [128, TAU, NP], BF16),
                            ("d_x8", [128, TAU, NV, 128], F8),
                            ("d_macc", [128, TAU, NP], BF16),
                            ("d_skip", [128, NP], F32),
                            ("d_rawE", [128, NP + 2], F32)]:
            taps[nm] = nc.dram_tensor(nm, shp, dt, kind="ExternalOutput").ap()

    with tile.TileContext(nc) as tc:
        with (
            tc.tile_pool(name="persist", bufs=1) as pp,
            tc.tile_pool(name="dram", bufs=1, space="DRAM") as dram,
        ):
            # ---- persistent tiles ----
            hcm = pp.tile([128, TAU, NP], BF16)       # f*g, channel-major (rows 126,127 zero)
            macc = pp.tile([128, TAU, NP], BF16)      # residual + mixprop accumulator
            skip_acc = pp.tile([128, NP], F32)        # skip0+skip1 partial
            rawE_sum = pp.tile([128, NP + 2], F32)    # combined rawE | stats
            x8_all = pp.tile([128, TAU, NV, 128], F8) # hcm transposed, fp8
            g00_t = pp.tile([128, NV, NP], F8)        # first hop matrix, preloaded
            w1T_t = pp.tile([CC, TAU, 128], BF16)
            wC_t = pp.tile([128, 8, 128], BF16)
            wET_t = pp.tile([128, TAU, 128], BF16)
            brv = pp.tile([128, 1], F32)
            wEs_t = pp.tile([128, 1], F32)
            b01_t = pp.tile([128, 1], F32)
            we1_t = pp.tile([128, 128], BF16)
            be1_t = pp.tile([128, 1], F32)
            we2_t = pp.tile([128, OUT], BF16)
            be2_t = pp.tile([OUT, 1], F32)
            whT_t = pp.tile([OUT, 1], BF16)
            bh_t = pp.tile([1, 1], F32)
            tmask_t = pp.tile([128, TAU], F32)
            sums_t = pp.tile([128, TAU], F32)
            sqs_t = pp.tile([128, TAU], F32)
            stats_p = pp.tile([128, 2], F32)
            ones_t = pp.tile([128, 128], F32)

            nc.gpsimd.dma_start(g00_t[:], gk_d[0][0][:])
            nc.vector.memset(hcm[:], 0.0)
            nc.vector.memset(macc[:, :, N:NP], 0.0)
            nc.vector.memset(ones_t[:], 1.0)

            # collective DRAM staging
            cc1_in = dram.tile([128, NP], F32)
            cc1_out = dram.tile([256, NP], F32)
            ccq_in = [dram.tile([128, NP + 2], F32, name=f"ccqi{q}")
                      for q in range(NQ)]
            ccq_out = [dram.tile([256, NP + 2], F32, name=f"ccqo{q}")
                       for q in range(NQ)]

            # ================= stage A =================
            with tc.tile_pool(name="stageA", bufs=1) as pa, \
                 tc.tile_pool(name="stag", bufs=2) as pstag:
                xh = pa.tile([128, TLOC, NP], BF16)
                H0 = pa.tile([128, TLOC, NP], BF16)
                xlo14 = pa.tile([TLOC, NP], BF16)       # [t, v] on 14 partitions
                ws_hi_t = pa.tile([128, 128], BF16)
                ws_lo_t = pa.tile([1, 128], BF16)
                w0_hi_t = pa.tile([128, TLOC, 128], BF16)
                w0_lo_t = pa.tile([TLOC, 128], BF16)
                wf_t = pa.tile([128, K, CC], BF16)
                wg_t = pa.tile([128, K, CC], BF16)
                bfv_t = pa.tile([CC, 1], F32)
                bgv_t = pa.tile([CC, 1], F32)
                for t_, d_ in [(ws_hi_t, wsT_hi), (ws_lo_t, wsT_lo),
                               (wf_t, wfT), (wg_t, wgT), (bfv_t, bf_v),
                               (bgv_t, bg_v), (w0_hi_t, w0T_hi),
                               (w0_lo_t, w0T_lo), (xlo14, x_lo)]:
                    nc.sync.dma_start(t_[:], d_[:])
                for tp_ in range(TLOC):
                    nc.sync.dma_start(xh[:, tp_, :], x_hi[:, tp_, :])

                for t_, d_ in [(w1T_t, w1T), (wC_t, wCT),
                               (wET_t, wET), (brv, b_resid_v), (wEs_t, wEsum_v),
                               (b01_t, b01_v), (we1_t, we1T), (be1_t, be1_v),
                               (we2_t, we2T), (be2_t, be2_v), (whT_t, whT),
                               (bh_t, bh_v), (tmask_t, tmask)]:
                    nc.gpsimd.dma_start(t_[:], d_[:])

                # start conv (H0) interleaved with filt/gate, per t'.
                # Interleaving keeps the PE continuously fed (p-state ramp).
                with tc.tile_pool(name="psA1", bufs=3, space="PSUM") as psA1, \
                     tc.tile_pool(name="psA2", bufs=4, space="PSUM") as psA2:

                    def fg_step(tau):
                        fs = pstag.tile([CC, NP], BF16, tag="fs")
                        gs = pstag.tile([CC, NP], BF16, tag="gs")
                        for dst, w_t, func, bias in ((fs, wf_t, AF.Tanh, bfv_t),
                                                     (gs, wg_t, AF.Sigmoid,
                                                      bgv_t)):
                            for vo, vl in VCH:
                                psb = psA2.tile([CC, 512], F32, tag="ps_fg")
                                for k in range(K):
                                    nc.tensor.matmul(psb[:, 0:vl], w_t[:, k, :],
                                                     H0[:, tau + k, vo:vo + vl],
                                                     start=(k == 0),
                                                     stop=(k == K - 1))
                                nc.scalar.activation(dst[:, vo:vo + vl],
                                                     psb[:, 0:vl], func,
                                                     bias=bias[:], scale=1.0)
                        nc.vector.tensor_tensor(hcm[0:CC, tau, :], fs[:], gs[:],
                                                op=ALU.mult)

                    for tp_ in range(TLOC):
                        stg = pstag.tile([1, NP], BF16, tag="xlo_stage")
                        nc.sync.dma_start(stg[:], x_lo[tp_:tp_ + 1, :])
                        for i, (vo, vl) in enumerate(VCH):
                            psum = psA1.tile([128, 512], F32, tag="ps_start")
                            nc.tensor.matmul(psum[:, 0:vl], ws_hi_t[:],
                                             xh[:, tp_, vo:vo + vl],
                                             start=True, stop=False)
                            nc.tensor.matmul(psum[:, 0:vl], ws_lo_t[:],
                                             stg[:, vo:vo + vl],
                                             start=False, stop=True)
                            if i % 2 == 0:
                                nc.vector.tensor_copy(H0[:, tp_, vo:vo + vl],
                                                      psum[:, 0:vl])
                            else:
                                nc.scalar.activation(H0[:, tp_, vo:vo + vl],
                                                     psum[:, 0:vl], AF.Copy)
                        if tp_ >= K - 1:
                            tau = tp_ - (K - 1)
                            fg_step(tau)
                            # residual (+ biases) into macc for this tau, so
                            # H0's space frees promptly for mixprop tiles
                            nc.vector.tensor_scalar_add(macc[:, tau, 0:N],
                                                        H0[:, tau + 2, 0:N],
                                                        brv[:])
                    # skip0: contract (c, t) chunk-sequential; c=128 via K=14
                    for i, (vo, vl) in enumerate(VCH):
                        s0ps = psA1.tile([128, 512], F32, tag="ps_start",
                                         name=f"s0ps{i}")
                        for tp_ in range(TLOC):
                            nc.tensor.matmul(s0ps[:, 0:vl], w0_hi_t[:, tp_, :],
                                             xh[:, tp_, vo:vo + vl],
                                             start=(tp_ == 0), stop=False)
                        nc.tensor.matmul(s0ps[:, 0:vl], w0_lo_t[:],
                                         xlo14[:, vo:vo + vl],
                                         start=False, stop=True)
                        nc.vector.tensor_copy(skip_acc[:, vo:vo + vl],
                                              s0ps[:, 0:vl])

                # skip1 conv partial (contract c,tau over local range)
                with tc.tile_pool(name="psA2s", bufs=2, space="PSUM") as psA2s:
                    for vo, vl in VCH:
                        psum = psA2s.tile([128, 512], F32, tag="ps_s1")
                        for tau in range(TAU):
                            nc.tensor.matmul(psum[:, 0:vl], w1T_t[:, tau, :],
                                             hcm[0:CC, tau, vo:vo + vl],
                                             start=(tau == 0), stop=(tau == TAU - 1))
                        nc.vector.tensor_tensor(skip_acc[:, vo:vo + vl],
                                                skip_acc[:, vo:vo + vl],
                                                psum[:, 0:vl], op=ALU.add)

            # ================= mixprop (powers of A, fp8 DoubleRow) ========
            with tc.tile_pool(name="mxg", bufs=1) as mxg, \
                 tc.tile_pool(name="mxu", bufs=1) as mxu, \
                 tc.tile_pool(name="mxr", bufs=2) as mxr, \
                 tc.tile_pool(name="mxT", bufs=2) as mxT, \
                 tc.tile_pool(name="psU", bufs=2, space="PSUM") as psU, \
                 tc.tile_pool(name="psC", bufs=2, space="PSUM") as psC:
                # g slot loads for mp=0 (k=1,2) BEFORE the cc1 collective so the
                # transfers overlap it on the in-order gpsimd queue
                g12_mp0 = []
                for k in (1, 2):
                    g = mxg.tile([128, NV, NP], F8, tag=f"g{k}", name=f"g{k}_0")
                    nc.gpsimd.dma_start(g[:], gk_d[0][k][:])
                    g12_mp0.append(g)

                for mp in range(2):
                    if mp == 0:
                        gs_t = [g00_t] + g12_mp0
                    else:
                        gs_t = []
                        for k in range(3):
                            g = mxg.tile([128, NV, NP], F8, tag=f"g{k}",
                                         name=f"g{k}_1")
                            nc.gpsimd.dma_start(g[:], gk_d[1][k][:])
                            gs_t.append(g)
                    for q in range(NQ):
                        u8 = mxu.tile([128, 3, 4, NP], BF16, tag="u8",
                                      name=f"u8_{mp}_{q}")
                        if mp == 0:
                            for ti in range(4):
                                t = 4 * q + ti
                                xT = mxT.tile([128, NV, 128], BF16, tag="xT")
                                nc.sync.dma_start_transpose(xT[:], hcm[:, t, :])
                                if ti % 2 == 0:
                                    nc.scalar.activation(x8_all[:, t, :, :],
                                                         xT[:], AF.Copy)
                                else:
                                    nc.vector.tensor_copy(x8_all[:, t, :, :],
                                                          xT[:])
                        for k in range(3):
                            for ti in range(4):
                                t = 4 * q + ti
                                pu = psU.tile([128, NP], F32, tag="pu")
                                for vo, vl in VCH:
                                    for j in range(5):
                                        nc.tensor.matmul(
                                            pu[:, vo:vo + vl],
                                            x8_all[:, t, 2 * j:2 * j + 2, :],
                                            gs_t[k][:, 2 * j:2 * j + 2, vo:vo + vl],
                                            start=(j == 0), stop=(j == 4),
                                            perf_mode=DR)
                                if (k + ti) % 2 == 0:
                                    nc.vector.tensor_copy(u8[:, k, ti, :], pu[:])
                                else:
                                    nc.scalar.activation(u8[:, k, ti, :], pu[:],
                                                         AF.Copy)
                        # conv1x1: C0 hcm + sum_k Ck u_k, accumulate into macc
                        for ti in range(4):
                            t = 4 * q + ti
                            for ci, (vo, vl) in enumerate(VCH):
                                pc = psC.tile([128, 512], F32, tag="pc")
                                nc.tensor.matmul(pc[:, 0:vl], wC_t[:, 4 * mp, :],
                                                 hcm[:, t, vo:vo + vl],
                                                 start=True, stop=False)
                                for k in range(3):
                                    nc.tensor.matmul(
                                        pc[:, 0:vl], wC_t[:, 4 * mp + 1 + k, :],
                                        u8[:, k, ti, vo:vo + vl],
                                        start=False, stop=(k == 2))
                                hi = min(vo + vl, N)
                                nc.vector.tensor_tensor(
                                    macc[:, t, vo:hi], macc[:, t, vo:hi],
                                    pc[:, 0:hi - vo], op=ALU.add)
                        if mp == 1:
                            # macc rows of q final: stats + rawE_q + collective
                            for ti in range(4):
                                t = 4 * q + ti
                                nc.vector.reduce_sum(sums_t[:, t:t + 1],
                                                     macc[:, t, :],
                                                     axis=mybir.AxisListType.X)
                                scr = mxT.tile([128, NP], BF16, tag="sq_scr")
                                nc.scalar.activation(scr[:], macc[:, t, :],
                                                     AF.Square,
                                                     accum_out=sqs_t[:, t:t + 1])
                            rq = mxr.tile([128, NP], F32, tag="rq")
                            for vo, vl in VCH:
                                psum = psC.tile([128, 512], F32, tag="pc",
                                                name="ps_rEq")
                                for ti in range(4):
                                    nc.tensor.matmul(
                                        psum[:, 0:vl], wET_t[:, 4 * q + ti, :],
                                        macc[:, 4 * q + ti, vo:vo + vl],
                                        start=(ti == 0), stop=(ti == 3))
                                nc.vector.tensor_copy(rq[:, vo:vo + vl],
                                                      psum[:, 0:vl])
                            nc.gpsimd.dma_start(ccq_in[q][:, 0:NP], rq[:])
                            if q == NQ - 1:
                                # layernorm partial stats appended to last cc
                                msum = mxr.tile([128, TAU], F32, tag="msum")
                                nc.vector.tensor_tensor(msum[:], sums_t[:],
                                                        tmask_t[:], op=ALU.mult)
                                nc.vector.reduce_sum(stats_p[:, 0:1], msum[:],
                                                     axis=mybir.AxisListType.X)
                                nc.vector.tensor_tensor(msum[:], sqs_t[:],
                                                        tmask_t[:], op=ALU.mult)
                                nc.vector.reduce_sum(stats_p[:, 1:2], msum[:],
                                                     axis=mybir.AxisListType.X)
                                nc.gpsimd.dma_start(ccq_in[q][:, NP:NP + 2],
                                                    stats_p[:])
                            nc.gpsimd.collective_compute(
                                "AllGather", ALU.bypass,
                                ins=[ccq_in[q].opt()], outs=[ccq_out[q].opt()],
                                replica_groups=[[0, 1], [2, 3], [4, 5], [6, 7]])
                            # combine halves (accumulating across q too)
                            if q == 0:
                                nc.gpsimd.dma_start(rawE_sum[:, 0:NP],
                                                    ccq_out[q][0:128, 0:NP])
                            else:
                                nc.gpsimd.dma_start(rawE_sum[:, 0:NP],
                                                    ccq_out[q][0:128, 0:NP],
                                                    accum_op=ALU.add)
                            nc.gpsimd.dma_start(rawE_sum[:, 0:NP],
                                                ccq_out[q][128:256, 0:NP],
                                                accum_op=ALU.add)
                            if q == NQ - 1:
                                nc.gpsimd.dma_start(
                                    rawE_sum[:, NP:NP + 2],
                                    ccq_out[q][0:128, NP:NP + 2])
                                nc.gpsimd.dma_start(
                                    rawE_sum[:, NP:NP + 2],
                                    ccq_out[q][128:256, NP:NP + 2],
                                    accum_op=ALU.add)
                    if mp == 0:
                        # pairwise AllGather of skip01 partials, emitted after
                        # mp=0 so its sync-engine completion wait sits behind
                        # the hcm transposes in the sync queue
                        nc.gpsimd.dma_start(cc1_in[:], skip_acc[:])
                        nc.gpsimd.collective_compute(
                            "AllGather", ALU.bypass,
                            ins=[cc1_in.opt()], outs=[cc1_out.opt()],
                            replica_groups=[[0, 1], [2, 3], [4, 5], [6, 7]])

            if debug_taps:
                nc.gpsimd.dma_start(taps["d_hcm"][:], hcm[:])
                nc.gpsimd.dma_start(taps["d_x8"][:], x8_all[:])
                nc.gpsimd.dma_start(taps["d_macc"][:], macc[:])
                nc.gpsimd.dma_start(taps["d_skip"][:], skip_acc[:])
            # ================= layernorm scalars + end stage =================
            with tc.tile_pool(name="late", bufs=1) as pl, \
                 tc.tile_pool(name="psL", bufs=1, space="PSUM") as ps:
                if debug_taps:
                    nc.gpsimd.dma_start(taps["d_rawE"][:], rawE_sum[:])
                # combine skip AllGather halves
                nc.gpsimd.dma_start(skip_acc[:], cc1_out[0:128, :])
                nc.gpsimd.dma_start(skip_acc[:], cc1_out[128:256, :],
                                    accum_op=ALU.add)

                # layernorm scalars: partition sum broadcast via ones-matmul
                st_r = pl.tile([128, 2], F32)
                pst = ps.tile([128, 2], F32, tag="ps_st")
                nc.tensor.matmul(pst[:], ones_t[:], rawE_sum[:, NP:NP + 2],
                                 start=True, stop=True)
                nc.vector.tensor_copy(st_r[:], pst[:])
                mv = pl.tile([128, 1], F32)
                msqv = pl.tile([128, 1], F32)
                varv = pl.tile([128, 1], F32)
                m2v = pl.tile([128, 1], F32)
                svv = pl.tile([128, 1], F32)
                rv = pl.tile([128, 1], F32)
                rmv = pl.tile([128, 1], F32)
                bias_c = pl.tile([128, 1], F32)
                nc.vector.tensor_scalar_mul(mv[:], st_r[:, 0:1], 1.0 / CNT)
                nc.vector.tensor_scalar_mul(msqv[:], st_r[:, 1:2], 1.0 / CNT)
                nc.vector.tensor_tensor(m2v[:], mv[:], mv[:], op=ALU.mult)
                nc.vector.tensor_scalar(varv[:], msqv[:], m2v[:], EPS,
                                        op0=ALU.subtract, op1=ALU.add)
                nc.scalar.sqrt(svv[:], varv[:])
                nc.vector.reciprocal(rv[:], svv[:])
                nc.vector.tensor_scalar(rmv[:], rv[:], mv[:], -1.0,
                                        op0=ALU.mult, op1=ALU.mult)
                # bias_c = b01 - r*m*wEsum
                nc.vector.scalar_tensor_tensor(bias_c[:], wEs_t[:], rmv[:],
                                               b01_t[:], ALU.mult, ALU.add)
                # skip_pre = skip01 + r*rawE ; relu with bias
                skip_pre = pl.tile([128, NP], F32)
                nc.vector.scalar_tensor_tensor(skip_pre[:], rawE_sum[:, 0:NP],
                                               rv[:], skip_acc[:],
                                               ALU.mult, ALU.add)
                rsk = pl.tile([128, NP], BF16)
                nc.vector.tensor_scalar(rsk[:], skip_pre[:], bias_c[:], 0.0,
                                        op0=ALU.add, op1=ALU.max)

                # end convs + head, chunk-pipelined across engines
                o1 = pl.tile([128, NP], BF16)
                o2 = pl.tile([OUT, NP], BF16)
                y_sb = pl.tile([1, NP], F32)
                ps1 = ps.tile([128, 1536], F32, tag="ps_e1")
                ps2 = ps.tile([OUT, 1536], F32, tag="ps_e2")
                psh = ps.tile([1, 1536], F32, tag="ps_e1", name="psh")
                for vo, vl in VCH:
                    nc.tensor.matmul(ps1[:, vo:vo + vl], we1_t[:],
                                     rsk[:, vo:vo + vl], start=True, stop=True)
                    nc.scalar.activation(o1[:, vo:vo + vl], ps1[:, vo:vo + vl],
                                         AF.Relu, bias=be1_t[:], scale=1.0)
                    nc.tensor.matmul(ps2[:, vo:vo + vl], we2_t[:],
                                     o1[:, vo:vo + vl], start=True, stop=True)
                    nc.vector.tensor_scalar_add(o2[:, vo:vo + vl],
                                                ps2[:, vo:vo + vl], be2_t[:])
                    nc.tensor.matmul(psh[:, vo:vo + vl], whT_t[:],
                                     o2[:, vo:vo + vl], start=True, stop=True)
                nc.scalar.activation(y_sb[:], psh[:, 0:NP], AF.Sigmoid,
                                     bias=bh_t[:], scale=1.0)
                nc.gpsimd.dma_start(y[:], y_sb[:])

    nc.compile()
    return nc


def _norm_adj(a):
    an = a + np.eye(N, dtype=np.float64)
    return an / an.sum(axis=1, keepdims=True)


def _gk_pack(ak):
    """SADJ * (A^k).T zero-padded to [NP, NP] -> [128, NV, NP] fp8."""
    gp = np.zeros((NP, NP), dtype=np.float32)
    gp[:N, :N] = SADJ * ak.T.astype(np.float32)
    gp = np.clip(gp, -240, 240)
    return gp.reshape(NV, 128, NP).transpose(1, 0, 2).astype(f8)


def _coefs(w):
    W = [w[:, i * CC:(i + 1) * CC] for i in range(4)]
    C0 = W[0] + 0.5 * (W[1] + W[2] + W[3])
    C1 = 0.5 * W[1] + 0.25 * W[2] + 0.125 * W[3]
    C2 = 0.25 * W[2] + 0.125 * W[3]
    C3 = 0.125 * W[3]
    return C0, C1, C2, C3


def _prep_inputs(inputs):
    x = np.asarray(inputs["x"], np.float32)
    adj = np.asarray(inputs["adj"], np.float32)
    w_start = np.asarray(inputs["w_start"], np.float32)
    b_start = np.asarray(inputs["b_start"], np.float32)
    w_filt = np.asarray(inputs["w_filt"], np.float32)[:, :, 0, :]
    b_filt = np.asarray(inputs["b_filt"], np.float32)
    w_gate = np.asarray(inputs["w_gate"], np.float32)[:, :, 0, :]
    b_gate = np.asarray(inputs["b_gate"], np.float32)
    w_skip0 = np.asarray(inputs["w_skip0"], np.float32)[:, :, 0, :]
    b_skip0 = np.asarray(inputs["b_skip0"], np.float32)
    w_skip1 = np.asarray(inputs["w_skip1"], np.float32)[:, :, 0, :]
    b_skip1 = np.asarray(inputs["b_skip1"], np.float32)
    w_mp1 = np.asarray(inputs["w_mp1"], np.float32)
    b_mp1 = np.asarray(inputs["b_mp1"], np.float32)
    w_mp2 = np.asarray(inputs["w_mp2"], np.float32)
    b_mp2 = np.asarray(inputs["b_mp2"], np.float32)
    w_skipE = np.asarray(inputs["w_skipE"], np.float32)[:, :, 0, :]
    b_skipE = np.asarray(inputs["b_skipE"], np.float32)
    w_end1 = np.asarray(inputs["w_end1"], np.float32)
    b_end1 = np.asarray(inputs["b_end1"], np.float32)
    w_end2 = np.asarray(inputs["w_end2"], np.float32)
    b_end2 = np.asarray(inputs["b_end2"], np.float32)
    w_head = np.asarray(inputs["w_head"], np.float32)
    b_head = np.asarray(inputs["b_head"], np.float32)

    shared = {}
    for mp, a0 in ((0, adj), (1, adj.T)):
        an = _norm_adj(a0.astype(np.float64))
        ak = an
        for k in range(3):
            shared[f"g{mp}{k}"] = _gk_pack(ak)
            if k < 2:
                ak = ak @ an

    # folded conv1x1 matrices: wCT[:, 4*mp + k, :] = C_k(mp).T (scaled)
    wC = np.zeros((128, 8, 128), np.float32)
    for mp, w in ((0, w_mp1), (1, w_mp2)):
        C = _coefs(w)
        wC[:CC, 4 * mp + 0, :] = C[0].T
        for k in range(1, 4):
            wC[:CC, 4 * mp + k, :] = C[k].T / SADJ
    shared["wCT"] = wC.astype(bf16)

    wsT = w_start.T  # [129, 128]
    shared.update({
        "wsT_hi": wsT[:128].astype(bf16),
        "wsT_lo": wsT[128:129].astype(bf16),
        "wfT": w_filt.transpose(1, 2, 0).astype(bf16),
        "wgT": w_gate.transpose(1, 2, 0).astype(bf16),
        "bf_v": (b_filt + w_filt.sum(2) @ b_start).reshape(CC, 1).astype(np.float32),
        "bg_v": (b_gate + w_gate.sum(2) @ b_start).reshape(CC, 1).astype(np.float32),
        "b_resid_v": (b_start + b_mp1 + b_mp2).reshape(128, 1).astype(np.float32),
        "wEsum_v": w_skipE.sum((1, 2)).reshape(128, 1).astype(np.float32),
        "b01_v": (b_skip0 + b_skip1 + b_skipE).reshape(128, 1).astype(np.float32),
        "we1T": w_end1.T.astype(bf16),
        "be1_v": b_end1.reshape(128, 1).astype(np.float32),
        "we2T": w_end2.T.astype(bf16),
        "be2_v": b_end2.reshape(OUT, 1).astype(np.float32),
        "whT": w_head.T.astype(bf16),
        "bh_v": b_head.reshape(1, 1).astype(np.float32),
    })

    in_maps = []
    for core in range(8):
        b, th = core // 2, core % 2
        t_lo = 0 if th == 0 else TAU
        # x slice [129, 1280, TLOC] zero-padded in nodes and t
        xp = np.zeros((C_IN, TLOC, NP), np.float32)
        t_hi = min(t_lo + TLOC, T)
        xp[:, 0:t_hi - t_lo, :N] = x[b, :, :, t_lo:t_hi].transpose(0, 2, 1)
        # skip0 weight slots aligned to local t: core owns t range
        w0T = np.zeros((C_IN, TLOC, 128), np.float32)
        own_lo, own_hi = (0, 13) if th == 0 else (13, T)
        for tp_ in range(TLOC):
            tg = t_lo + tp_
            if own_lo <= tg < own_hi:
                w0T[:, tp_, :] = w_skip0[:, :, tg].T
        # skip1 / skipE weight slots aligned to local tau
        w1Ta = np.zeros((CC, TAU, 128), np.float32)
        wETa = np.zeros((128, TAU, 128), np.float32)
        for tau in range(TAU):
            tg = t_lo + tau
            if tg < T1:
                w1Ta[:, tau, :] = w_skip1[:, :, tg].T
                wETa[:, tau, :] = w_skipE[:, :, tg].T
        tm = np.ones((128, TAU), np.float32)
        if th == 1:
            tm[:, T1 - TAU:] = 0.0  # tau slots beyond T1 are padding
        m = dict(shared)
        m["x_hi"] = xp[:128].astype(bf16)
        m["x_lo"] = xp[128].astype(bf16)
        m["w0T_hi"] = w0T[:128].astype(bf16)
        m["w0T_lo"] = w0T[128].astype(bf16)
        m["w1T"] = w1Ta.astype(bf16)
        m["wET"] = wETa.astype(bf16)
        m["tmask"] = tm
        in_maps.append(m)
    return in_maps


def kernel(**inputs):
    if "nc" not in _CACHE:
        _CACHE["nc"] = _build_program()
    nc = _CACHE["nc"]
    in_maps = _prep_inputs(inputs)
    res = bass_utils.run_bass_kernel_spmd(nc, in_maps, core_ids=list(range(8)))
    out = np.empty((B, N), np.float32)
    for b in range(B):
        out[b] = res.results[2 * b]["y"][0, :N]
    return out


# revision 32
# speedup vs baseline: 1.3111x; 1.0006x over previous
"""Trainium2 Bass kernel for MTGNN temporal classifier (single layer).

Self-contained: takes FULL inputs as numpy arrays, shards across 8 NeuronCores
(batch x time-half), runs one SPMD Bass/Tile program, gathers the full output.

Sharding: core = 2*b + th  (b in 0..3 batches, th in 0..1 time-halves).

Mixprop is computed via the powers-of-A refactor: since the channel conv1x1
commutes with node hops,  out = sum_k C_k A^k x  with C_k folded host-side.
The A^k are precomputed on host, scaled by 256 and quantized to fp8e4, so the
dominant N x N hop GEMMs run in fp8 DoubleRow (double-pumped) mode with
x (channel-major hcm transposed once per time step) as the fp8 stationary.
Hop outputs land directly channel-major, eliminating per-hop transposes and
blends.  LayerNorm is folded analytically into the skipE convolution; the
collectives are pairwise AllGathers: skip01 early, and rawE per row-chunk q
(overlapped with compute) with [stats] appended to the last one.
"""

import numpy as np
import ml_dtypes

import concourse.bass as bass
import concourse.tile as tile
import concourse.bass_isa as bass_isa
from concourse import bacc, mybir
from concourse import bass_utils

BF16 = mybir.dt.bfloat16
F32 = mybir.dt.float32
F8 = mybir.dt.float8e4
bf16 = ml_dtypes.bfloat16
f8 = ml_dtypes.float8_e4m3
AF = mybir.ActivationFunctionType
ALU = mybir.AluOpType
DR = mybir.MatmulPerfMode.DoubleRow

# problem dims
B, C_IN, N, T = 4, 129, 1259, 25
RC, CC, SC, EC, OUT = 128, 126, 128, 128, 64
K = 3
T1 = T - (K - 1)          # 23
NP = 1280                 # padded node count
NV = NP // 128            # 10 node blocks
TAU = 12                  # local output time steps per core (incl. 1 pad on th=1)
TLOC = TAU + 2            # 14 local input time steps
VCH = [(0, 512), (512, 512), (1024, 256)]   # v chunks (full NP)
CNT = float(RC * N * T1)  # layernorm element count per batch
EPS = 1e-5
NQ = 3                    # row-chunk count (4 tau each)
SADJ = 256.0              # fp8 scale on A^k

_CACHE = {}


def _build_program(debug_taps=False):
    nc = bacc.Bacc("TRN2", target_bir_lowering=False, debug=False, num_devices=8)

    def din(name, shape, dt=BF16):
        return nc.dram_tensor(name, shape, dt, kind="ExternalInput").ap()

    x_hi = din("x_hi", [128, TLOC, NP])
    x_lo = din("x_lo", [TLOC, NP])          # channel 128, [t, v]
    gk_d = [[din(f"g{mp}{k}", [128, NV, NP], F8) for k in range(3)]
            for mp in range(2)]             # SADJ * (A^k).T padded, fp8
    wsT_hi = din("wsT_hi", [128, 128])
    wsT_lo = din("wsT_lo", [1, 128])
    w0T_hi = din("w0T_hi", [128, TLOC, 128])
    w0T_lo = din("w0T_lo", [TLOC, 128])
    wfT = din("wfT", [128, K, CC])
    wgT = din("wgT", [128, K, CC])
    bf_v = din("bf_v", [CC, 1], F32)
    bg_v = din("bg_v", [CC, 1], F32)
    w1T = din("w1T", [CC, TAU, 128])
    wCT = din("wCT", [128, 8, 128])         # folded conv mats, transposed
    b_resid_v = din("b_resid_v", [128, 1], F32)
    wET = din("wET", [128, TAU, 128])
    wEsum_v = din("wEsum_v", [128, 1], F32)
    b01_v = din("b01_v", [128, 1], F32)
    we1T = din("we1T", [128, 128])
    be1_v = din("be1_v", [128, 1], F32)
    we2T = din("we2T", [128, OUT])
    be2_v = din("be2_v", [OUT, 1], F32)
    whT = din("whT", [OUT, 1])
    bh_v = din("bh_v", [1, 1], F32)
    tmask = din("tmask", [128, TAU], F32)
    y = nc.dram_tensor("y", [1, NP], F32, kind="ExternalOutput").ap()
    taps = {}
    if debug_taps:
        for nm, shp, dt in [("d_hcm", [128, TAU, NP], BF16),
                            ("d_x8", [128, TAU, NV, 128], F8),
                            ("d_macc", [128, TAU, NP], BF16),
                            ("d_skip", [128, NP], F32),
                            ("d_rawE", [128, NP + 2], F32)]:
            taps[nm] = nc.dram_tensor(nm, shp, dt, kind="ExternalOutput").ap()

    with tile.TileContext(nc) as tc:
        with (
            tc.tile_pool(name="persist", bufs=1) as pp,
            tc.tile_pool(name="dram", bufs=1, space="DRAM") as dram,
        ):
            # ---- persistent tiles ----
            hcm = pp.tile([128, TAU, NP], BF16)       # f*g, channel-major (rows 126,127 zero)
            macc = pp.tile([128, TAU, NP], BF16)      # residual + mixprop accumulator
            skip_acc = pp.tile([128, NP], F32)        # skip0+skip1 partial
            rawE_sum = pp.tile([128, NP + 2], F32)    # combined rawE | stats
            x8_all = pp.tile([128, TAU, NV, 128], F8) # hcm transposed, fp8
            g00_t = pp.tile([128, NV, NP], F8)        # first hop matrix, preloaded
            w1T_t = pp.tile([CC, TAU, 128], BF16)
            wC_t = pp.tile([128, 8, 128], BF16)
            wET_t = pp.tile([128, TAU, 128], BF16)
            brv = pp.tile([128, 1], F32)
            wEs_t = pp.tile([128, 1], F32)
            b01_t = pp.tile([128, 1], F32)
            we1_t = pp.tile([128, 128], BF16)
            be1_t = pp.tile([128, 1], F32)
            we2_t = pp.tile([128, OUT], BF16)
            be2_t = pp.tile([OUT, 1], F32)
            whT_t = pp.tile([OUT, 1], BF16)
            bh_t = pp.tile([1, 1], F32)
            tmask_t = pp.tile([128, TAU], F32)
            sums_t = pp.tile([128, TAU], F32)
            sqs_t = pp.tile([128, TAU], F32)
            stats_p = pp.tile([128, 2], F32)
            ones_t = pp.tile([128, 128], F32)

            nc.gpsimd.dma_start(g00_t[:], gk_d[0][0][:])
            nc.vector.memset(hcm[:], 0.0)
            nc.vector.memset(macc[:, :, N:NP], 0.0)
            nc.vector.memset(ones_t[:], 1.0)

            # collective DRAM staging
            cc1_in = dram.tile([128, NP], F32)
            cc1_out = dram.tile([256, NP], F32)
            ccq_in = [dram.tile([128, NP + 2], F32, name=f"ccqi{q}")
                      for q in range(NQ)]
            ccq_out = [dram.tile([256, NP + 2], F32, name=f"ccqo{q}")
                       for q in range(NQ)]

            # ================= stage A =================
            with tc.tile_pool(name="stageA", bufs=1) as pa, \
                 tc.tile_pool(name="stag", bufs=2) as pstag:
                xh = pa.tile([128, TLOC, NP], BF16)
                H0 = pa.tile([128, TLOC, NP], BF16)
                xlo14 = pa.tile([TLOC, NP], BF16)       # [t, v] on 14 partitions
                ws_hi_t = pa.tile([128, 128], BF16)
                ws_lo_t = pa.tile([1, 128], BF16)
                w0_hi_t = pa.tile([128, TLOC, 128], BF16)
                w0_lo_t = pa.tile([TLOC, 128], BF16)
                wf_t = pa.tile([128, K, CC], BF16)
                wg_t = pa.tile([128, K, CC], BF16)
                bfv_t = pa.tile([CC, 1], F32)
                bgv_t = pa.tile([CC, 1], F32)
                for t_, d_ in [(ws_hi_t, wsT_hi), (ws_lo_t, wsT_lo),
                               (wf_t, wfT), (wg_t, wgT), (bfv_t, bf_v),
                               (bgv_t, bg_v), (w0_hi_t, w0T_hi),
                               (w0_lo_t, w0T_lo), (xlo14, x_lo)]:
                    nc.sync.dma_start(t_[:], d_[:])
                for tp_ in range(TLOC):
                    eng = nc.scalar if tp_ % 2 == 0 else nc.sync
                    eng.dma_start(xh[:, tp_, :], x_hi[:, tp_, :])

                for t_, d_ in [(w1T_t, w1T), (wC_t, wCT),
                               (wET_t, wET), (brv, b_resid_v), (wEs_t, wEsum_v),
                               (b01_t, b01_v), (we1_t, we1T), (be1_t, be1_v),
                               (we2_t, we2T), (be2_t, be2_v), (whT_t, whT),
                               (bh_t, bh_v), (tmask_t, tmask)]:
                    nc.gpsimd.dma_start(t_[:], d_[:])

                # start conv (H0) interleaved with filt/gate, per t'.
                # Interleaving keeps the PE continuously fed (p-state ramp).
                with tc.tile_pool(name="psA1", bufs=3, space="PSUM") as psA1, \
                     tc.tile_pool(name="psA2", bufs=4, space="PSUM") as psA2:

                    def fg_step(tau):
                        fs = pstag.tile([CC, NP], BF16, tag="fs")
                        gs = pstag.tile([CC, NP], BF16, tag="gs")
                        for dst, w_t, func, bias in ((fs, wf_t, AF.Tanh, bfv_t),
                                                     (gs, wg_t, AF.Sigmoid,
                                                      bgv_t)):
                            for vo, vl in VCH:
                                psb = psA2.tile([CC, 512], F32, tag="ps_fg")
                                for k in range(K):
                                    nc.tensor.matmul(psb[:, 0:vl], w_t[:, k, :],
                                                     H0[:, tau + k, vo:vo + vl],
                                                     start=(k == 0),
                                                     stop=(k == K - 1))
                                nc.scalar.activation(dst[:, vo:vo + vl],
                                                     psb[:, 0:vl], func,
                                                     bias=bias[:], scale=1.0)
                        nc.vector.tensor_tensor(hcm[0:CC, tau, :], fs[:], gs[:],
                                                op=ALU.mult)

                    for tp_ in range(TLOC):
                        stg = pstag.tile([1, NP], BF16, tag="xlo_stage")
                        nc.sync.dma_start(stg[:], x_lo[tp_:tp_ + 1, :])
                        for i, (vo, vl) in enumerate(VCH):
                            psum = psA1.tile([128, 512], F32, tag="ps_start")
                            nc.tensor.matmul(psum[:, 0:vl], ws_hi_t[:],
                                             xh[:, tp_, vo:vo + vl],
                                             start=True, stop=False)
                            nc.tensor.matmul(psum[:, 0:vl], ws_lo_t[:],
                                             stg[:, vo:vo + vl],
                                             start=False, stop=True)
                            nc.vector.tensor_copy(H0[:, tp_, vo:vo + vl],
                                                  psum[:, 0:vl])
                        if tp_ >= K - 1:
                            tau = tp_ - (K - 1)
                            fg_step(tau)
                            # residual (+ biases) into macc for this tau, so
                            # H0's space frees promptly for mixprop tiles
                            nc.vector.tensor_scalar_add(macc[:, tau, 0:N],
                                                        H0[:, tau + 2, 0:N],
                                                        brv[:])
                    # skip0: contract (c, t) chunk-sequential; c=128 via K=14
                    for i, (vo, vl) in enumerate(VCH):
                        s0ps = psA1.tile([128, 512], F32, tag="ps_start",
                                         name=f"s0ps{i}")
                        for tp_ in range(TLOC):
                            nc.tensor.matmul(s0ps[:, 0:vl], w0_hi_t[:, tp_, :],
                                             xh[:, tp_, vo:vo + vl],
                                             start=(tp_ == 0), stop=False)
                        nc.tensor.matmul(s0ps[:, 0:vl], w0_lo_t[:],
                                         xlo14[:, vo:vo + vl],
                                         start=False, stop=True)
                        nc.vector.tensor_copy(skip_acc[:, vo:vo + vl],
                                              s0ps[:, 0:vl])

                # skip1 conv partial (contract c,tau over local range)
                with tc.tile_pool(name="psA2s", bufs=2, space="PSUM") as psA2s:
                    for vo, vl in VCH:
                        psum = psA2s.tile([128, 512], F32, tag="ps_s1")
                        for tau in range(TAU):
                            nc.tensor.matmul(psum[:, 0:vl], w1T_t[:, tau, :],
                                             hcm[0:CC, tau, vo:vo + vl],
                                             start=(tau == 0), stop=(tau == TAU - 1))
                        nc.vector.tensor_tensor(skip_acc[:, vo:vo + vl],
                                                skip_acc[:, vo:vo + vl],
                                                psum[:, 0:vl], op=ALU.add)

            # ================= mixprop (powers of A, fp8 DoubleRow) ========
            with tc.tile_pool(name="mxg", bufs=1) as mxg, \
                 tc.tile_pool(name="mxu", bufs=1) as mxu, \
                 tc.tile_pool(name="mxr", bufs=2) as mxr, \
                 tc.tile_pool(name="mxT", bufs=2) as mxT, \
                 tc.tile_pool(name="psU", bufs=2, space="PSUM") as psU, \
                 tc.tile_pool(name="psC", bufs=2, space="PSUM") as psC:
                # g slot loads for mp=0 (k=1,2) BEFORE the cc1 collective so the
                # transfers overlap it on the in-order gpsimd queue
                g12_mp0 = []
                for k in (1, 2):
                    g = mxg.tile([128, NV, NP], F8, tag=f"g{k}", name=f"g{k}_0")
                    nc.gpsimd.dma_start(g[:], gk_d[0][k][:])
                    g12_mp0.append(g)

                # all hcm transposes + fp8 quantizes up front: DMA transposes
                # are hard-serialized against collectives by the scheduler, so
                # they must all precede the cc1 trigger
                for t in range(TAU):
                    xT = mxT.tile([128, NV, 128], BF16, tag="xT")
                    nc.sync.dma_start_transpose(xT[:], hcm[:, t, :])
                    if t % 2 == 0:
                        nc.scalar.activation(x8_all[:, t, :, :], xT[:], AF.Copy)
                    else:
                        nc.vector.tensor_copy(x8_all[:, t, :, :], xT[:])

                # pairwise AllGather of skip01 partials (combined in the late
                # stage); after the transposes to avoid serializing them
                nc.gpsimd.dma_start(cc1_in[:], skip_acc[:])
                nc.gpsimd.collective_compute(
                    "AllGather", ALU.bypass,
                    ins=[cc1_in.opt()], outs=[cc1_out.opt()],
                    replica_groups=[[0, 1], [2, 3], [4, 5], [6, 7]])

                for mp in range(2):
                    if mp == 0:
                        gs_t = [g00_t] + g12_mp0
                    else:
                        gs_t = []
                        for k in range(3):
                            g = mxg.tile([128, NV, NP], F8, tag=f"g{k}",
                                         name=f"g{k}_1")
                            nc.gpsimd.dma_start(g[:], gk_d[1][k][:])
                            gs_t.append(g)
                    for q in range(NQ):
                        u8 = mxu.tile([128, 3, 4, NP], BF16, tag="u8",
                                      name=f"u8_{mp}_{q}")
                        for k in range(3):
                            for ti in range(4):
                                t = 4 * q + ti
                                pu = psU.tile([128, NP], F32, tag="pu")
                                for vo, vl in VCH:
                                    for j in range(5):
                                        nc.tensor.matmul(
                                            pu[:, vo:vo + vl],
                                            x8_all[:, t, 2 * j:2 * j + 2, :],
                                            gs_t[k][:, 2 * j:2 * j + 2, vo:vo + vl],
                                            start=(j == 0), stop=(j == 4),
                                            perf_mode=DR)
                                if (k + ti) % 2 == 0:
                                    nc.vector.tensor_copy(u8[:, k, ti, :], pu[:])
                                else:
                                    nc.scalar.activation(u8[:, k, ti, :], pu[:],
                                                         AF.Copy)
                        # conv1x1: C0 hcm + sum_k Ck u_k, accumulate into macc
                        for ti in range(4):
                            t = 4 * q + ti
                            for ci, (vo, vl) in enumerate(VCH):
                                pc = psC.tile([128, 512], F32, tag="pc")
                                nc.tensor.matmul(pc[:, 0:vl], wC_t[:, 4 * mp, :],
                                                 hcm[:, t, vo:vo + vl],
                                                 start=True, stop=False)
                                for k in range(3):
                                    nc.tensor.matmul(
                                        pc[:, 0:vl], wC_t[:, 4 * mp + 1 + k, :],
                                        u8[:, k, ti, vo:vo + vl],
                                        start=False, stop=(k == 2))
                                hi = min(vo + vl, N)
                                nc.vector.tensor_tensor(
                                    macc[:, t, vo:hi], macc[:, t, vo:hi],
                                    pc[:, 0:hi - vo], op=ALU.add)
                        if mp == 1:
                            # macc rows of q final: stats + rawE_q + collective
                            for ti in range(4):
                                t = 4 * q + ti
                                nc.vector.reduce_sum(sums_t[:, t:t + 1],
                                                     macc[:, t, :],
                                                     axis=mybir.AxisListType.X)
                                scr = mxT.tile([128, NP], BF16, tag="sq_scr")
                                nc.scalar.activation(scr[:], macc[:, t, :],
                                                     AF.Square,
                                                     accum_out=sqs_t[:, t:t + 1])
                            rq = mxr.tile([128, NP], F32, tag="rq")
                            for vo, vl in VCH:
                                psum = psC.tile([128, 512], F32, tag="pc",
                                                name="ps_rEq")
                                for ti in range(4):
                                    nc.tensor.matmul(
                                        psum[:, 0:vl], wET_t[:, 4 * q + ti, :],
                                        macc[:, 4 * q + ti, vo:vo + vl],
                                        start=(ti == 0), stop=(ti == 3))
                                nc.vector.tensor_copy(rq[:, vo:vo + vl],
                                                      psum[:, 0:vl])
                            nc.gpsimd.dma_start(ccq_in[q][:, 0:NP], rq[:])
                            if q == NQ - 1:
                                # layernorm partial stats appended to last cc
                                msum = mxr.tile([128, TAU], F32, tag="msum")
                                nc.vector.tensor_tensor(msum[:], sums_t[:],
                                                        tmask_t[:], op=ALU.mult)
                                nc.vector.reduce_sum(stats_p[:, 0:1], msum[:],
                                                     axis=mybir.AxisListType.X)
                                nc.vector.tensor_tensor(msum[:], sqs_t[:],
                                                        tmask_t[:], op=ALU.mult)
                                nc.vector.reduce_sum(stats_p[:, 1:2], msum[:],
                                                     axis=mybir.AxisListType.X)
                                nc.gpsimd.dma_start(ccq_in[q][:, NP:NP + 2],
                                                    stats_p[:])
                            nc.gpsimd.collective_compute(
                                "AllGather", ALU.bypass,
                                ins=[ccq_in[q].opt()], outs=[ccq_out[q].opt()],
                                replica_groups=[[0, 1], [2, 3], [4, 5], [6, 7]])
                            # combine halves (accumulating across q too)
                            if q == 0:
                                nc.gpsimd.dma_start(rawE_sum[:, 0:NP],
                                                    ccq_out[q][0:128, 0:NP])
                            else:
                                nc.gpsimd.dma_start(rawE_sum[:, 0:NP],
                                                    ccq_out[q][0:128, 0:NP],
                                                    accum_op=ALU.add)
                            nc.gpsimd.dma_start(rawE_sum[:, 0:NP],
                                                ccq_out[q][128:256, 0:NP],
                                                accum_op=ALU.add)
                            if q == NQ - 1:
                                nc.gpsimd.dma_start(
                                    rawE_sum[:, NP:NP + 2],
                                    ccq_out[q][0:128, NP:NP + 2])
                                nc.gpsimd.dma_start(
                                    rawE_sum[:, NP:NP + 2],
                                    ccq_out[q][128:256, NP:NP + 2],
                                    accum_op=ALU.add)


            if debug_taps:
                nc.gpsimd.dma_start(taps["d_hcm"][:], hcm[:])
                nc.gpsimd.dma_start(taps["d_x8"][:], x8_all[:])
                nc.gpsimd.dma_start(taps["d_macc"][:], macc[:])
                nc.gpsimd.dma_start(taps["d_skip"][:], skip_acc[:])
            # ================= layernorm scalars + end stage =================
            with tc.tile_pool(name="late", bufs=1) as pl, \
                 tc.tile_pool(name="psL", bufs=1, space="PSUM") as ps:
                if debug_taps:
                    nc.gpsimd.dma_start(taps["d_rawE"][:], rawE_sum[:])
                # combine skip AllGather halves
                nc.gpsimd.dma_start(skip_acc[:], cc1_out[0:128, :])
                nc.gpsimd.dma_start(skip_acc[:], cc1_out[128:256, :],
                                    accum_op=ALU.add)

                # layernorm scalars: partition sum broadcast via ones-matmul
                st_r = pl.tile([128, 2], F32)
                pst = ps.tile([128, 2], F32, tag="ps_st")
                nc.tensor.matmul(pst[:], ones_t[:], rawE_sum[:, NP:NP + 2],
                                 start=True, stop=True)
                nc.vector.tensor_copy(st_r[:], pst[:])
                mv = pl.tile([128, 1], F32)
                msqv = pl.tile([128, 1], F32)
                varv = pl.tile([128, 1], F32)
                m2v = pl.tile([128, 1], F32)
                svv = pl.tile([128, 1], F32)
                rv = pl.tile([128, 1], F32)
                rmv = pl.tile([128, 1], F32)
                bias_c = pl.tile([128, 1], F32)
                nc.vector.tensor_scalar_mul(mv[:], st_r[:, 0:1], 1.0 / CNT)
                nc.vector.tensor_scalar_mul(msqv[:], st_r[:, 1:2], 1.0 / CNT)
                nc.vector.tensor_tensor(m2v[:], mv[:], mv[:], op=ALU.mult)
                nc.vector.tensor_scalar(varv[:], msqv[:], m2v[:], EPS,
                                        op0=ALU.subtract, op1=ALU.add)
                nc.scalar.sqrt(svv[:], varv[:])
                nc.vector.reciprocal(rv[:], svv[:])
                nc.vector.tensor_scalar(rmv[:], rv[:], mv[:], -1.0,
                                        op0=ALU.mult, op1=ALU.mult)
                # bias_c = b01 - r*m*wEsum
                nc.vector.scalar_tensor_tensor(bias_c[:], wEs_t[:], rmv[:],
                                               b01_t[:], ALU.mult, ALU.add)
                # skip_pre = skip01 + r*rawE ; relu with bias
                skip_pre = pl.tile([128, NP], F32)
                nc.vector.scalar_tensor_tensor(skip_pre[:], rawE_sum[:, 0:NP],
                                               rv[:], skip_acc[:],
                                               ALU.mult, ALU.add)
                rsk = pl.tile([128, NP], BF16)
                nc.vector.tensor_scalar(rsk[:], skip_pre[:], bias_c[:], 0.0,
                                        op0=ALU.add, op1=ALU.max)

                # end convs + head, chunk-pipelined across engines
                o1 = pl.tile([128, NP], BF16)
                o2 = pl.tile([OUT, NP], BF16)
                y_sb = pl.tile([1, NP], F32)
                ps1 = ps.tile([128, 1536], F32, tag="ps_e1")
                ps2 = ps.tile([OUT, 1536], F32, tag="ps_e2")
                psh = ps.tile([1, 1536], F32, tag="ps_e1", name="psh")
                for vo, vl in VCH:
                    nc.tensor.matmul(ps1[:, vo:vo + vl], we1_t[:],
                                     rsk[:, vo:vo + vl], start=True, stop=True)
                    nc.scalar.activation(o1[:, vo:vo + vl], ps1[:, vo:vo + vl],
                                         AF.Relu, bias=be1_t[:], scale=1.0)
                    nc.tensor.matmul(ps2[:, vo:vo + vl], we2_t[:],
                                     o1[:, vo:vo + vl], start=True, stop=True)
                    nc.vector.tensor_scalar_add(o2[:, vo:vo + vl],
                                                ps2[:, vo:vo + vl], be2_t[:])
                    nc.tensor.matmul(psh[:, vo:vo + vl], whT_t[:],
                                     o2[:, vo:vo + vl], start=True, stop=True)
                nc.scalar.activation(y_sb[:], psh[:, 0:NP], AF.Sigmoid,
                                     bias=bh_t[:], scale=1.0)
                nc.gpsimd.dma_start(y[:], y_sb[:])

    nc.compile()
    return nc


def _norm_adj(a):
    an = a + np.eye(N, dtype=np.float64)
    return an / an.sum(axis=1, keepdims=True)


def _gk_pack(ak):
    """SADJ * (A^k).T zero-padded to [NP, NP] -> [128, NV, NP] fp8."""
    gp = np.zeros((NP, NP), dtype=np.float32)
    gp[:N, :N] = SADJ * ak.T.astype(np.float32)
    gp = np.clip(gp, -240, 240)
    return gp.reshape(NV, 128, NP).transpose(1, 0, 2).astype(f8)


def _coefs(w):
    W = [w[:, i * CC:(i + 1) * CC] for i in range(4)]
    C0 = W[0] + 0.5 * (W[1] + W[2] + W[3])
    C1 = 0.5 * W[1] + 0.25 * W[2] + 0.125 * W[3]
    C2 = 0.25 * W[2] + 0.125 * W[3]
    C3 = 0.125 * W[3]
    return C0, C1, C2, C3


def _prep_inputs(inputs):
    x = np.asarray(inputs["x"], np.float32)
    adj = np.asarray(inputs["adj"], np.float32)
    w_start = np.asarray(inputs["w_start"], np.float32)
    b_start = np.asarray(inputs["b_start"], np.float32)
    w_filt = np.asarray(inputs["w_filt"], np.float32)[:, :, 0, :]
    b_filt = np.asarray(inputs["b_filt"], np.float32)
    w_gate = np.asarray(inputs["w_gate"], np.float32)[:, :, 0, :]
    b_gate = np.asarray(inputs["b_gate"], np.float32)
    w_skip0 = np.asarray(inputs["w_skip0"], np.float32)[:, :, 0, :]
    b_skip0 = np.asarray(inputs["b_skip0"], np.float32)
    w_skip1 = np.asarray(inputs["w_skip1"], np.float32)[:, :, 0, :]
    b_skip1 = np.asarray(inputs["b_skip1"], np.float32)
    w_mp1 = np.asarray(inputs["w_mp1"], np.float32)
    b_mp1 = np.asarray(inputs["b_mp1"], np.float32)
    w_mp2 = np.asarray(inputs["w_mp2"], np.float32)
    b_mp2 = np.asarray(inputs["b_mp2"], np.float32)
    w_skipE = np.asarray(inputs["w_skipE"], np.float32)[:, :, 0, :]
    b_skipE = np.asarray(inputs["b_skipE"], np.float32)
    w_end1 = np.asarray(inputs["w_end1"], np.float32)
    b_end1 = np.asarray(inputs["b_end1"], np.float32)
    w_end2 = np.asarray(inputs["w_end2"], np.float32)
    b_end2 = np.asarray(inputs["b_end2"], np.float32)
    w_head = np.asarray(inputs["w_head"], np.float32)
    b_head = np.asarray(inputs["b_head"], np.float32)

    shared = {}
    for mp, a0 in ((0, adj), (1, adj.T)):
        an = _norm_adj(a0.astype(np.float64))
        ak = an
        for k in range(3):
            shared[f"g{mp}{k}"] = _gk_pack(ak)
            if k < 2:
                ak = ak @ an

    # folded conv1x1 matrices: wCT[:, 4*mp + k, :] = C_k(mp).T (scaled)
    wC = np.zeros((128, 8, 128), np.float32)
    for mp, w in ((0, w_mp1), (1, w_mp2)):
        C = _coefs(w)
        wC[:CC, 4 * mp + 0, :] = C[0].T
        for k in range(1, 4):
            wC[:CC, 4 * mp + k, :] = C[k].T / SADJ
    shared["wCT"] = wC.astype(bf16)

    wsT = w_start.T  # [129, 128]
    shared.update({
        "wsT_hi": wsT[:128].astype(bf16),
        "wsT_lo": wsT[128:129].astype(bf16),
        "wfT": w_filt.transpose(1, 2, 0).astype(bf16),
        "wgT": w_gate.transpose(1, 2, 0).astype(bf16),
        "bf_v": (b_filt + w_filt.sum(2) @ b_start).reshape(CC, 1).astype(np.float32),
        "bg_v": (b_gate + w_gate.sum(2) @ b_start).reshape(CC, 1).astype(np.float32),
        "b_resid_v": (b_start + b_mp1 + b_mp2).reshape(128, 1).astype(np.float32),
        "wEsum_v": w_skipE.sum((1, 2)).reshape(128, 1).astype(np.float32),
        "b01_v": (b_skip0 + b_skip1 + b_skipE).reshape(128, 1).astype(np.float32),
        "we1T": w_end1.T.astype(bf16),
        "be1_v": b_end1.reshape(128, 1).astype(np.float32),
        "we2T": w_end2.T.astype(bf16),
        "be2_v": b_end2.reshape(OUT, 1).astype(np.float32),
        "whT": w_head.T.astype(bf16),
        "bh_v": b_head.reshape(1, 1).astype(np.float32),
    })

    in_maps = []
    for core in range(8):
        b, th = core // 2, core % 2
        t_lo = 0 if th == 0 else TAU
        # x slice [129, 1280, TLOC] zero-padded in nodes and t
        xp = np.zeros((C_IN, TLOC, NP), np.float32)
        t_hi = min(t_lo + TLOC, T)
        xp[:, 0:t_hi - t_lo, :N] = x[b, :, :, t_lo:t_hi].transpose(0, 2, 1)
        # skip0 weight slots aligned to local t: core owns t range
        w0T = np.zeros((C_IN, TLOC, 128), np.float32)
        own_lo, own_hi = (0, 13) if th == 0 else (13, T)
        for tp_ in range(TLOC):
            tg = t_lo + tp_
            if own_lo <= tg < own_hi:
                w0T[:, tp_, :] = w_skip0[:, :, tg].T
        # skip1 / skipE weight slots aligned to local tau
        w1Ta = np.zeros((CC, TAU, 128), np.float32)
        wETa = np.zeros((128, TAU, 128), np.float32)
        for tau in range(TAU):
            tg = t_lo + tau
            if tg < T1:
                w1Ta[:, tau, :] = w_skip1[:, :, tg].T
                wETa[:, tau, :] = w_skipE[:, :, tg].T
        tm = np.ones((128, TAU), np.float32)
        if th == 1:
            tm[:, T1 - TAU:] = 0.0  # tau slots beyond T1 are padding
        m = dict(shared)
        m["x_hi"] = xp[:128].astype(bf16)
        m["x_lo"] = xp[128].astype(bf16)
        m["w0T_hi"] = w0T[:128].astype(bf16)
        m["w0T_lo"] = w0T[128].astype(bf16)
        m["w1T"] = w1Ta.astype(bf16)
        m["wET"] = wETa.astype(bf16)
        m["tmask"] = tm
        in_maps.append(m)
    return in_maps


def kernel(**inputs):
    if "nc" not in _CACHE:
        _CACHE["nc"] = _build_program()
    nc = _CACHE["nc"]
    in_maps = _prep_inputs(inputs)
    res = bass_utils.run_bass_kernel_spmd(nc, in_maps, core_ids=list(range(8)))
    out = np.empty((B, N), np.float32)
    for b in range(B):
        out[b] = res.results[2 * b]["y"][0, :N]
    return out
